# revision 28
# baseline (speedup 1.0000x reference)
"""Trainium2 Bass kernel for the two-branch GCN (nn_GCNN) -- v3.

Math per branch (A includes self-loops and symmetric deg^-1/2 norm):
  S = A @ X; C = S @ W + b; L = (1-a)relu(C) (a*C pooled term is ~7e-6,
  dropped); per-graph mean pool; MLPs; sigmoid head.

v3 structure (vs the per-128-dst-tile v1):
  * The symmetric norm dinv[src]*dinv[dst] is SEPARABLE: dinv[src]*8 is
    folded into the gathered x rows on the host (xd = x*dinv*8 in fp8),
    and dinv[dst] rides the existing relu activation (func(in*scale+b))
    with the GCN bias row pre-scaled by sqrt(deg[dst]) so that
    relu(dinv_d * (S~W + sqrtdeg_d*b)) == relu(C).  The one-hot scatter
    matrix is then a PURE 0/1 indicator -> single-op is_equal expansion
    on the DVE (half the cost of the fused is_equal*norm form).
  * Destination tiles are processed in PAIRS sharing one slot space:
    each distinct source row is gathered once per 256 destinations
    instead of once per 128 -- ~1.2x fewer gathered rows (the gather is
    the kernel's bottleneck at ~6.7ns per 1KB row).  The aggregation
    keeps the v1 matmul orientation (lhsT=hot[slot, dst128],
    rhs=G[slot, f512], fp8 DoubleRow) which amortizes the contraction
    stream over 512 output columns.
Per 128-dst tile the downstream pipeline (S psum -> scaled copy ->
transpose -> fp8 S^T -> GEMM vs W(x32) -> scaled relu -> pooling
matmuls -> branch MLP -> head) matches v1.
"""

import numpy as np
import ml_dtypes

import concourse.bacc as bacc
import concourse.mybir as mybir
import concourse.tile as tile
from concourse.bass_utils import run_bass_kernel_spmd
from concourse.masks import make_identity

BF16 = mybir.dt.bfloat16
FP8E4 = mybir.dt.float8e4
F32 = mybir.dt.float32
I16 = mybir.dt.int16
P = 128
N_CORES = 8
N_GRAPHS = 32
GPC = N_GRAPHS // N_CORES  # graphs per core
GS_TILES = 2               # dst tiles per slot group (pair)
GS = GS_TILES * P

DIMS = dict(n_nodes=10000, f_in=1024, fp=128, hf1=256, hf2=64)

GDT_NP = ml_dtypes.float8_e4m3
HOT_SCALE = 8.0   # folded into xd rows (S~ carries x8)
WG_SCALE = 32.0   # prescale on the GCN weight for fp8 (C carries x256)
LEAK_ALPHA = 0.01


# ---------------------------------------------------------------- host prep


def _branch_prep(x, edge_index, batch, n_nodes, f_in):
    """Per-branch host preprocessing. Returns per-core arrays + static meta."""
    src = np.asarray(edge_index[0], dtype=np.int64)
    dst = np.asarray(edge_index[1], dtype=np.int64)
    batch = np.asarray(batch, dtype=np.int64)

    deg = np.bincount(dst, minlength=n_nodes).astype(np.float64) + 1.0
    dinv = (1.0 / np.sqrt(deg)).astype(np.float32).astype(np.float64)

    # append self loops
    allsrc = np.concatenate([src, np.arange(n_nodes, dtype=np.int64)])
    alldst = np.concatenate([dst, np.arange(n_nodes, dtype=np.int64)])

    # node ranges per core (batch is sorted)
    bounds = np.searchsorted(batch, np.arange(0, N_GRAPHS + 1, GPC))
    npad = int(np.ceil(max(1, np.diff(bounds).max()) / P) * P)
    t_d = npad // P
    n_groups = (t_d + GS_TILES - 1) // GS_TILES
    nch = [min(GS_TILES, t_d - g * GS_TILES) for g in range(n_groups)]

    edge_core = batch[alldst] // GPC

    # per (core, group): slots = distinct src; passes per slot =
    # max over the group's tiles of the (src, tile) edge multiplicity
    core_g = [[None] * n_groups for _ in range(N_CORES)]
    slot_cnt = np.zeros((N_CORES, n_groups), dtype=np.int64)
    npass = [[np.zeros(1, np.int64)] * n_groups for _ in range(N_CORES)]
    for c in range(N_CORES):
        m = edge_core == c
        es, ed = allsrc[m], alldst[m]
        ld = ed - bounds[c]
        g_of = ld // GS
        for g in range(n_groups):
            mm = g_of == g
            if not mm.any():
                continue
            s_, l_ = es[mm], ld[mm] - g * GS
            chunk, colc = l_ // P, l_ % P
            uniq, sinv = np.unique(s_, return_inverse=True)
            k = len(uniq)
            sc = sinv * GS_TILES + chunk
            sc_cnt = np.bincount(sc, minlength=k * GS_TILES)
            passes = sc_cnt.reshape(k, GS_TILES).max(axis=1)
            slot_cnt[c, g] = k
            kmax = int(passes.max())
            npass[c][g] = np.array([(passes > p).sum() for p in range(kmax)],
                                   np.int64)
            core_g[c][g] = (uniq, sinv, chunk, colc, passes)

    t_s = np.maximum(1, np.ceil(slot_cnt.max(axis=0) / P).astype(np.int64))
    t_s = ((t_s + 1) // 2) * 2  # even, for DoubleRow subtile pairs
    t0 = np.concatenate([[0], np.cumsum(t_s)])
    t_tot = int(t0[-1])
    km = [max(len(npass[c][g]) for c in range(N_CORES)) for g in range(n_groups)]
    nsub = []
    for g in range(n_groups):
        row = [int(t_s[g])]
        for p in range(1, km[g]):
            mx = 0
            for c in range(N_CORES):
                h = npass[c][g]
                if len(h) > p:
                    mx = max(mx, int(np.ceil(h[p] / P)))
            row.append(max(1, mx))
        nsub.append(row)
    toff = []
    acc = 0
    for g in range(n_groups):
        row = []
        for p in range(km[g]):
            row.append(acc)
            acc += nsub[g][p] * nch[g]
        toff.append(row)
    t_cols = acc

    # x rows pre-scaled by the SRC norm factor and the fp8 headroom scale
    xd = (
        np.ascontiguousarray(np.asarray(x, dtype=np.float32))
        * (dinv.astype(np.float32) * HOT_SCALE)[:, None]
    ).astype(GDT_NP)

    per_core = []
    for c in range(N_CORES):
        src_arr = np.zeros((t_tot, P), dtype=np.int32)
        dca = np.full((P, t_cols), -1.0, dtype=np.float32)
        for g in range(n_groups):
            if core_g[c][g] is None:
                continue
            uniq, sinv, chunk, colc, passes = core_g[c][g]
            k = len(uniq)
            order = np.argsort(-passes, kind="stable")  # slots by passes desc
            rank = np.empty(k, np.int64)
            rank[order] = np.arange(k)
            src_arr[int(t0[g]) + rank // P, rank % P] = uniq
            er = rank[sinv]
            key = er * GS_TILES + chunk
            eo = np.argsort(key, kind="stable")
            ks = key[eo]
            starts = np.searchsorted(ks, np.arange(k * GS_TILES))
            within = np.arange(len(ks)) - starts[ks]  # pass index per edge
            toff_arr = np.asarray(toff[g], np.int64)
            col = toff_arr[within] + (er[eo] // P) * nch[g] + chunk[eo]
            dca[er[eo] % P, col] = colc[eo].astype(np.float32)
        # dst-side norm tables: dd[p, d] = dinv[dst], dg[0, d*P+p] = 1/dinv
        dd = np.zeros((P, t_d), dtype=np.float32)
        dg = np.zeros((1, t_d * P), dtype=np.float32)
        nc_lo, nc_hi = bounds[c], bounds[c + 1]
        nloc = nc_hi - nc_lo
        dloc = dinv[nc_lo:nc_hi].astype(np.float32)
        ii = np.arange(nloc)
        dd[ii % P, ii // P] = dloc
        dg[0, ii] = 1.0 / dloc
        # pool matrix [t_d, P, GPC]; counts
        pm = np.zeros((t_d, P, GPC), dtype=ml_dtypes.bfloat16)
        loc_g = (batch[nc_lo:nc_hi] - c * GPC).astype(np.int64)
        pm[ii // P, ii % P, loc_g] = 1.0
        cnt = np.bincount(loc_g, minlength=GPC).astype(np.float64)
        ci = (
            (1.0 - LEAK_ALPHA)
            / (np.maximum(cnt, 1.0) * HOT_SCALE * WG_SCALE)
        ).astype(np.float32)
        blk16 = np.zeros((16, t_tot * 8), dtype=np.int16)
        for g in range(n_groups):
            ni = int(t_s[g]) * P
            blk = src_arr[int(t0[g]) : int(t0[g]) + int(t_s[g])].reshape(ni)
            blk16[
                np.arange(ni) % 16,
                int(t0[g]) * 8 + np.arange(ni) // 16,
            ] = blk.astype(np.int16)
        src16 = np.tile(blk16, (8, 1))
        per_core.append(
            {
                "src": src16,  # [P, t_tot*8] int16
                "dca": dca,
                "dd": dd,
                "dg": dg.astype(ml_dtypes.bfloat16),
                "pm": np.ascontiguousarray(
                    pm.transpose(1, 0, 2).reshape(P, t_d * GPC)
                ),
                "ci": np.broadcast_to(
                    np.tile(ci, f_in // P), (P, (f_in // P) * GPC)
                ).copy(),
            }
        )
    meta = {
        "t_d": t_d,
        "n_groups": n_groups,
        "nch": nch,
        "t_s": [int(v) for v in t_s],
        "t0": [int(v) for v in t0],
        "km": [int(v) for v in km],
        "nsub": [[int(v) for v in row] for row in nsub],
        "toff": [[int(v) for v in row] for row in toff],
        "t_cols": int(t_cols),
    }
    return xd, per_core, meta


def _ktile(w, f_in):
    """[f_in, n] -> [P, (f_in//P)*n] SBUF k-tile layout."""
    f, n = w.shape
    assert f == f_in
    return (
        np.ascontiguousarray(w)
        .reshape(f // P, P, n)
        .transpose(1, 0, 2)
        .reshape(P, (f // P) * n)
    )


def _balance_graphs(batch1, batch2):
    """Greedy 4-graph-per-core bin packing balancing combined node counts.

    The contiguous [4c, 4c+4) windows leave the unluckiest core ~1330
    nodes (an 11th dst tile and inflated shared max-over-cores subtile
    counts).  Balancing brings every core under 1280 nodes for both
    branches.  Returns the 32 original graph ids in their new (core-
    major) positions; callers permute nodes so the assignment is again
    contiguous and the existing prep/kernel work unchanged."""
    n1 = np.bincount(np.asarray(batch1, np.int64), minlength=N_GRAPHS)
    n2 = np.bincount(np.asarray(batch2, np.int64), minlength=N_GRAPHS)
    w = n1 + n2
    loads = np.zeros(N_CORES)
    sets = [[] for _ in range(N_CORES)]
    for g in np.argsort(-w, kind="stable"):
        full = np.array([len(s) >= GPC for s in sets])
        c = int(np.argmin(np.where(full, np.inf, loads)))
        sets[c].append(int(g))
        loads[c] += w[g]
    return [g for s in sets for g in sorted(s)]


def _permute_branch(x, edge_index, batch, newpos):
    """Renumber nodes so graphs appear in newpos order (contiguously)."""
    batch = np.asarray(batch, np.int64)
    pi = np.argsort(newpos[batch], kind="stable")   # new row -> old row
    inv = np.empty_like(pi)
    inv[pi] = np.arange(len(pi))                    # old row -> new row
    x_p = np.asarray(x)[pi]
    ei = inv[np.asarray(edge_index, np.int64)]
    batch_p = newpos[batch][pi]
    return x_p, ei, batch_p


def prep_inputs(inputs, dims):
    n_nodes, f_in = dims["n_nodes"], dims["f_in"]
    fp, hf1, hf2 = dims["fp"], dims["hf1"], dims["hf2"]

    gperm = _balance_graphs(inputs["pro1_batch"], inputs["pro2_batch"])
    newpos = np.empty(N_GRAPHS, np.int64)
    newpos[np.asarray(gperm)] = np.arange(N_GRAPHS)
    x1, e1, b1 = _permute_branch(
        inputs["pro1_x"], inputs["pro1_edge_index"], inputs["pro1_batch"], newpos
    )
    x2, e2, b2 = _permute_branch(
        inputs["pro2_x"], inputs["pro2_edge_index"], inputs["pro2_batch"], newpos
    )

    x1_bf, pc1, meta1 = _branch_prep(x1, e1, b1, n_nodes, f_in)
    x2_bf, pc2, meta2 = _branch_prep(x2, e2, b2, n_nodes, f_in)

    f32 = np.float32
    shared = {
        "xg1": x1_bf,
        "xg2": x2_bf,
        "wg1": (_ktile(np.asarray(inputs["Wg1"], f32), f_in) * WG_SCALE).astype(GDT_NP),
        "wg2": (_ktile(np.asarray(inputs["Wg2"], f32), f_in) * WG_SCALE).astype(GDT_NP),
        "bg1": (np.asarray(inputs["bg1"], f32)[None, :] * (HOT_SCALE * WG_SCALE)).astype(
            ml_dtypes.bfloat16
        ),
        "bg2": (np.asarray(inputs["bg2"], f32)[None, :] * (HOT_SCALE * WG_SCALE)).astype(
            ml_dtypes.bfloat16
        ),
        "wp1": _ktile(np.asarray(inputs["Wp1"], f32), f_in).astype(ml_dtypes.bfloat16),
        "wp2": _ktile(np.asarray(inputs["Wp2"], f32), f_in).astype(ml_dtypes.bfloat16),
        # leaky(C) = (1-a)relu(C) + a*C; the pooled a*C term is dropped
        # (~7e-6 of the output) except its free bias part: bp' = bp + a*Wp^T b
        "bp1": (
            np.asarray(inputs["bp1"], f32)
            + LEAK_ALPHA * (np.asarray(inputs["Wp1"], f32).T @ np.asarray(inputs["bg1"], f32))
        )[:, None],
        "bp2": (
            np.asarray(inputs["bp2"], f32)
            + LEAK_ALPHA * (np.asarray(inputs["Wp2"], f32).T @ np.asarray(inputs["bg2"], f32))
        )[:, None],
        "wf1": _ktile(np.asarray(inputs["Wf1"], f32), 2 * fp),
        "bf1": np.asarray(inputs["bf1"], f32).reshape(hf1 // P, P).T.copy(),
        "wf2": _ktile(np.asarray(inputs["Wf2"], f32), hf1),
        "bf2": np.asarray(inputs["bf2"], f32)[:, None],
        "wo": np.asarray(inputs["Wo"], f32),
        "bo": np.asarray(inputs["bo"], f32)[:, None],
    }
    shared["iot"] = np.tile(
        np.arange(P, dtype=np.float32)[None, :], (P, 1)
    ).astype(ml_dtypes.bfloat16)
    in_maps = []
    for c in range(N_CORES):
        m = dict(shared)
        for br, pc in (("1", pc1), ("2", pc2)):
            for k in ("src", "dca", "dd", "dg", "pm", "ci"):
                m[k + br] = pc[c][k]
        in_maps.append(m)
    meta = {"b1": meta1, "b2": meta2, "dims": dims, "gperm": gperm}
    return in_maps, meta


# ---------------------------------------------------------------- program


def _bias_leaky(nc, pool, out_ap, psum_ap, bias_col):
    """out = leaky_relu(psum + bias); bias_col is a per-partition [p,1] AP."""
    p, n = psum_ap.shape
    z = pool.tile([p, n], F32, tag="blz")
    nc.vector.tensor_scalar_add(out=z[:], in0=psum_ap, scalar1=bias_col)
    t = pool.tile([p, n], F32, tag="blt")
    nc.vector.tensor_scalar_mul(out=t[:], in0=z[:], scalar1=LEAK_ALPHA)
    nc.vector.tensor_tensor(out=out_ap, in0=z[:], in1=t[:], op=mybir.AluOpType.max)


def build_program(meta, loop_n=1):
    dims = meta["dims"]
    n_nodes, f_in = dims["n_nodes"], dims["f_in"]
    fp, hf1, hf2 = dims["fp"], dims["hf1"], dims["hf2"]
    CH = f_in // P  # k-chunks of gcn layer
    NH = (f_in + 511) // 512  # N-halves of 512
    NS = min(f_in, 512)
    CHH = NS // P  # k-chunks per half
    NP32 = CH * GPC
    DR = mybir.MatmulPerfMode.DoubleRow

    nc = bacc.Bacc(
        "TRN2",
        target_bir_lowering=False,
        debug=False,
        num_devices=N_CORES,
        num_swdge_queues=4,
    )

    def din(name, shape, dt):
        return nc.dram_tensor(name, list(shape), dt, kind="ExternalInput").ap()

    aps = {}
    for br in ("1", "2"):
        m = meta["b" + br]
        t_tot = m["t0"][-1]
        aps["xg" + br] = din("xg" + br, [n_nodes, f_in], FP8E4)
        aps["src" + br] = din("src" + br, [P, t_tot * 8], I16)
        aps["dca" + br] = din("dca" + br, [P, m["t_cols"]], F32)
        aps["dd" + br] = din("dd" + br, [P, m["t_d"]], F32)
        aps["dg" + br] = din("dg" + br, [1, m["t_d"] * P], BF16)
        aps["pm" + br] = din("pm" + br, [P, m["t_d"] * GPC], BF16)
        aps["ci" + br] = din("ci" + br, [P, CH * GPC], F32)
        aps["wg" + br] = din("wg" + br, [P, CH * f_in], FP8E4)
        aps["bg" + br] = din("bg" + br, [1, f_in], BF16)
        aps["wp" + br] = din("wp" + br, [P, CH * fp], BF16)
        aps["bp" + br] = din("bp" + br, [fp, 1], F32)
    aps["wf1"] = din("wf1", [P, (2 * fp // P) * hf1], F32)
    aps["bf1"] = din("bf1", [P, hf1 // P], F32)
    aps["wf2"] = din("wf2", [P, (hf1 // P) * hf2], F32)
    aps["bf2"] = din("bf2", [hf2, 1], F32)
    aps["wo"] = din("wo", [hf2, 1], F32)
    aps["bo"] = din("bo", [1, 1], F32)
    aps["iot"] = din("iot", [P, P], BF16)
    out_ap = nc.dram_tensor("out", [1, GPC], F32, kind="ExternalOutput").ap()

    SIG = mybir.ActivationFunctionType.Sigmoid

    with tile.TileContext(nc) as tc:
        with (
            tc.tile_pool(name="const", bufs=1) as cpool,
            tc.tile_pool(name="gp", bufs=8) as gpool,
            tc.tile_pool(name="hp", bufs=3) as hpool,
            tc.tile_pool(name="ip", bufs=1) as ipool,
            tc.tile_pool(name="sp", bufs=10) as spool,
            tc.tile_pool(name="tp", bufs=3) as tpool,
            tc.tile_pool(name="lp", bufs=2) as lpool,
            tc.tile_pool(name="acc", bufs=1) as apool,
            tc.tile_pool(name="spsum", bufs=3, space="PSUM") as spsum,
            tc.tile_pool(name="tpsum", bufs=2, space="PSUM") as tpsum,
            tc.tile_pool(name="cpsum", bufs=2, space="PSUM") as cpsum,
            tc.tile_pool(name="ppsum", bufs=1, space="PSUM") as ppsum,
        ):
            ident = cpool.tile([P, P], BF16)
            make_identity(nc, ident[:])

            idxt = {}
            pmt = {}
            dvt = {}

            def load_tables(br):
                t = ipool.tile(list(aps["src" + br].shape), I16, tag="idx" + br)
                nc.sync.dma_start(out=t[:], in_=aps["src" + br][:])
                idxt[br] = t
                t = ipool.tile(list(aps["pm" + br].shape), BF16, tag="pm" + br)
                nc.sync.dma_start(out=t[:], in_=aps["pm" + br][:])
                pmt[br] = t
                dc = ipool.tile(list(aps["dca" + br].shape), F32, tag="dca" + br)
                nc.sync.dma_start(out=dc[:], in_=aps["dca" + br][:])
                dd = ipool.tile(list(aps["dd" + br].shape), F32, tag="dd" + br)
                nc.sync.dma_start(out=dd[:], in_=aps["dd" + br][:])
                dg = ipool.tile(list(aps["dg" + br].shape), BF16, tag="dg" + br)
                nc.sync.dma_start(out=dg[:], in_=aps["dg" + br][:])
                dvt[br] = (dc, dd, dg)

            iot = cpool.tile([P, P], BF16, tag="iot")
            nc.sync.dma_start(out=iot[:], in_=aps["iot"][:])
            load_tables("1")

            wt = {}
            WDTYPES = dict(
                wg1=FP8E4, wg2=FP8E4, bg1=BF16, bg2=BF16,
                wp1=BF16, wp2=BF16,
                bp1=F32, bp2=F32, ci1=F32, ci2=F32,
                wf1=F32, bf1=F32, wf2=F32, bf2=F32, wo=F32, bo=F32,
            )

            def load_weights(names, eng=None):
                eng = eng or nc.scalar
                for name in names:
                    t = cpool.tile(list(aps[name].shape), WDTYPES[name], tag=name)
                    eng.dma_start(out=t[:], in_=aps[name][:])
                    wt[name] = t

            def emit_body():
                hbr = {}
                qrr = [0]
                for br in ("1", "2"):
                    m = meta["b" + br]
                    t_d, t_s, t0 = m["t_d"], m["t_s"], m["t0"]
                    n_groups, nch = m["n_groups"], m["nch"]
                    xg = aps["xg" + br]
                    pmb = pmt[br]
                    dca_t, dd_t, dg_t = dvt[br]

                    pq = ppsum.tile([P, 512], F32, tag="pps")
                    wgv = wt.get("wg" + br)
                    pending = []

                    def emit_downstream(g, nc_g, ssb_t):
                        wgv = wt["wg" + br][:].rearrange(
                            "p (k n) -> p k n", n=f_in
                        )
                        tsb_t = {}
                        for ch in range(nc_g):
                            for h in range(NH):
                                t_ps = tpsum.tile([P, NS], BF16, tag="tps",
                                                  name="t_ps")
                                s_sb = ssb_t[(ch, h)]
                                for ck in range(CHH):
                                    nc.tensor.transpose(
                                        t_ps[:, ck * P : (ck + 1) * P],
                                        s_sb[:, ck * P : (ck + 1) * P],
                                        ident[:],
                                    )
                                t_sb = tpool.tile(
                                    [P, NS], FP8E4, tag=f"tsb{ch}h{h}",
                                    name="t_sb",
                                )
                                nc.scalar.copy(out=t_sb[:], in_=t_ps[:])
                                tsb_t[(ch, h)] = t_sb
                        for ch in range(nc_g):
                            d = g * GS_TILES + ch
                            leak = lpool.tile([P, f_in], BF16, tag="leak",
                                              name="leak")
                            for ho in range(NH):
                                c_ps = cpsum.tile([P, NS], F32, tag="c",
                                                  name="c_ps")
                                for kk in range(0, CH, 2):
                                    tsv = tsb_t[(ch, kk // CHH)][:].rearrange(
                                        "p (k d) -> p k d", d=P
                                    )
                                    kl = kk % CHH
                                    nc.tensor.matmul(
                                        c_ps[:, :],
                                        lhsT=tsv[:, kl : kl + 2, :],
                                        rhs=wgv[:, kk : kk + 2, ho * NS : (ho + 1) * NS],
                                        start=(kk == 0),
                                        stop=False,
                                        perf_mode=DR,
                                    )
                                # bias row scaled by sqrt(deg[dst]) so the
                                # dinv[dst] relu scale restores +b exactly
                                nc.tensor.matmul(
                                    c_ps[:, :],
                                    lhsT=dg_t[:1, d * P : (d + 1) * P],
                                    rhs=wt["bg" + br][:1, ho * NS : (ho + 1) * NS],
                                    start=False,
                                    stop=True,
                                )
                                nc.scalar.activation(
                                    out=leak[:, ho * NS : (ho + 1) * NS],
                                    in_=c_ps[:],
                                    func=mybir.ActivationFunctionType.Relu,
                                    scale=dd_t[:, d : d + 1],
                                )
                                for ck in range(ho * CHH, ho * CHH + CHH):
                                    nc.tensor.matmul(
                                        pq[:, ck * GPC : (ck + 1) * GPC],
                                        lhsT=leak[:, ck * P : (ck + 1) * P],
                                        rhs=pmb[:, d * GPC : (d + 1) * GPC],
                                        start=(d == 0 and ck == 0),
                                        stop=(d == t_d - 1 and ck == CH - 1),
                                        skip_group_check=True,
                                    )

                    for g in range(n_groups):
                        if br == "1" and g == 0:
                            load_weights(("bg1", "ci1"), eng=nc.sync)
                        if br == "1" and g == 1:
                            load_tables("2")
                            load_weights((
                                "bg2", "ci2", "wg2", "wp1", "wp2",
                                "bp1", "bp2", "wf1", "bf1", "wf2",
                                "bf2", "wo", "bo",
                            ))
                        ts, tg0, nc_g = t_s[g], t0[g], nch[g]
                        # indicator one-hot expansion (single-op is_equal):
                        # hot[slot, (subtile, tile, dstcol)]
                        hott = hpool.tile([P, ts * GS], FP8E4, tag="hot")
                        o0 = m["toff"][g][0]
                        for j in range(ts):
                            for ch in range(nc_g):
                                ccol = o0 + j * nc_g + ch
                                nc.vector.tensor_scalar(
                                    out=hott[:, j * GS + ch * P : j * GS + (ch + 1) * P],
                                    in0=iot[:],
                                    scalar1=dca_t[:, ccol : ccol + 1],
                                    scalar2=None,
                                    op0=mybir.AluOpType.is_equal,
                                )
                        for pp in range(1, m["km"][g]):
                            op = m["toff"][g][pp]
                            for j in range(m["nsub"][g][pp]):
                                for ch in range(nc_g):
                                    ccol = op + j * nc_g + ch
                                    hx = lpool.tile([P, P], FP8E4, tag="hx")
                                    nc.vector.tensor_scalar(
                                        out=hx[:],
                                        in0=iot[:],
                                        scalar1=dca_t[:, ccol : ccol + 1],
                                        scalar2=None,
                                        op0=mybir.AluOpType.is_equal,
                                    )
                                    nc.vector.tensor_tensor(
                                        out=hott[
                                            :, j * GS + ch * P : j * GS + (ch + 1) * P
                                        ],
                                        in0=hott[
                                            :, j * GS + ch * P : j * GS + (ch + 1) * P
                                        ],
                                        in1=hx[:],
                                        op=mybir.AluOpType.add,
                                    )
                        gh = []
                        for j0 in range(0, ts, 8):
                            gn = min(8, ts - j0)
                            gt = gpool.tile([P, gn * f_in], FP8E4, tag="g")
                            nc.gpsimd.dma_gather(
                                out_ap=gt[:].rearrange(
                                    "p (t e) -> p t e", e=f_in
                                ),
                                in_ap=xg[:],
                                idxs_ap=idxt[br][
                                    :, (tg0 + j0) * 8 : (tg0 + j0 + gn) * 8
                                ],
                                num_idxs=gn * P,
                                num_idxs_reg=gn * P,
                                elem_size=f_in,
                                queue_num=qrr[0] % 4,
                            )
                            qrr[0] += 1
                            gh.append((j0, gn, gt))
                        if br == "1" and g == 0:
                            load_weights(("wg1",))

                        hot3 = hott[:].rearrange("p (t d) -> p t d", d=GS)
                        wgv = wt["wg" + br][:].rearrange("p (k n) -> p k n", n=f_in)
                        ssb_t = {}
                        for h in range(NH):
                            # S~ psum per tile of the pair, this feature half
                            s_ps = [
                                spsum.tile([P, NS], F32, tag="s", name="s_ps")
                                for _ in range(nc_g)
                            ]
                            for j0, gn, gt in gh:
                                gt3 = gt[:].rearrange(
                                    "p (t e) -> p t e", e=f_in
                                )
                                for jj in range(0, gn, 2):
                                    ja = j0 + jj
                                    for ch in range(nc_g):
                                        nc.tensor.matmul(
                                            s_ps[ch][:, :],
                                            lhsT=hot3[
                                                :, ja : ja + 2,
                                                ch * P : (ch + 1) * P,
                                            ],
                                            rhs=gt3[
                                                :, jj : jj + 2,
                                                h * NS : (h + 1) * NS,
                                            ],
                                            start=(ja == 0),
                                            stop=(ja == ts - 2),
                                            perf_mode=DR,
                                            skip_group_check=True,
                                        )
                            for ch in range(nc_g):
                                s_sb = spool.tile([P, NS], BF16, tag="ssb")
                                nc.scalar.copy(out=s_sb[:], in_=s_ps[ch][:])
                                ssb_t[(ch, h)] = s_sb
                        pending.append((g, nc_g, ssb_t))
                        # software pipelining: emit the previous pair's
                        # downstream (transpose -> GEMM -> relu -> pool) now,
                        # so its cross-engine latencies hide under this
                        # pair's aggregation matmuls
                        if len(pending) > 1:
                            emit_downstream(*pending.pop(0))

                    while pending:
                        emit_downstream(*pending.pop(0))

                    # poolacc = ci * pool(relu C);  h = Wp^T @ poolacc + bp'
                    pa = apool.tile([P, NP32], BF16, tag="poolacc" + br)
                    nc.vector.tensor_tensor(
                        out=pa[:],
                        in0=pq[:, :NP32],
                        in1=wt["ci" + br][:],
                        op=mybir.AluOpType.mult,
                    )
                    h_ps = ppsum.tile([P, GPC], F32, tag="pps")
                    for ck in range(CH):
                        nc.tensor.matmul(
                            h_ps[:, :],
                            lhsT=wt["wp" + br][:, ck * fp : (ck + 1) * fp],
                            rhs=pa[:, ck * GPC : (ck + 1) * GPC],
                            start=(ck == 0),
                            stop=(ck == CH - 1),
                        )
                    hb = apool.tile([fp, GPC], F32, tag="hbr" + br)
                    _bias_leaky(nc, apool, hb[:], h_ps[:fp, :], wt["bp" + br][:, :1])
                    hbr[br] = hb

                # head
                K1 = 2 * fp // P
                M1 = hf1 // P
                rhs_k = [hbr["1"], hbr["2"]]
                hh = apool.tile([P, M1 * GPC], F32, tag="hh")
                for mt in range(M1):
                    f_ps = ppsum.tile([P, GPC], F32, tag="pps")
                    for kk in range(K1):
                        nc.tensor.matmul(
                            f_ps[:, :],
                            lhsT=wt["wf1"][
                                :, kk * hf1 + mt * P : kk * hf1 + (mt + 1) * P
                            ],
                            rhs=rhs_k[kk][:, :],
                            start=(kk == 0),
                            stop=(kk == K1 - 1),
                        )
                    _bias_leaky(
                        nc, apool, hh[:, mt * GPC : (mt + 1) * GPC], f_ps[:, :],
                        wt["bf1"][:, mt : mt + 1],
                    )
                g_ps = ppsum.tile([hf2, GPC], F32, tag="pps")
                for kk in range(M1):
                    nc.tensor.matmul(
                        g_ps[:, :],
                        lhsT=wt["wf2"][:, kk * hf2 : (kk + 1) * hf2],
                        rhs=hh[:, kk * GPC : (kk + 1) * GPC],
                        start=(kk == 0),
                        stop=(kk == M1 - 1),
                    )
                h3 = apool.tile([hf2, GPC], F32, tag="h3")
                _bias_leaky(nc, apool, h3[:], g_ps[:], wt["bf2"][:, :1])
                o_ps = ppsum.tile([1, GPC], F32, tag="pps")
                nc.tensor.matmul(
                    o_ps[:, :], lhsT=wt["wo"][:, :1], rhs=h3[:, :],
                    start=True, stop=True,
                )
                o_sb = apool.tile([1, GPC], F32, tag="o_sb")
                nc.scalar.activation(
                    out=o_sb[:], in_=o_ps[:], func=SIG, bias=wt["bo"][:1, :1]
                )
                nc.sync.dma_start(out=out_ap[:], in_=o_sb[:])

            if loop_n > 1:
                with tc.For_i(0, loop_n, 1):
                    emit_body()
            else:
                emit_body()

    nc.compile()
    return nc


# ---------------------------------------------------------------- entry


_CACHE = {}


def _program_key(meta):
    def bkey(m):
        return (
            m["t_d"], m["n_groups"], tuple(m["nch"]), tuple(m["t_s"]),
            tuple(m["km"]),
            tuple(tuple(r) for r in m["nsub"]),
        )
    return (bkey(meta["b1"]), bkey(meta["b2"]))


def get_program(meta):
    key = _program_key(meta)
    if key not in _CACHE:
        _CACHE[key] = build_program(meta)
    return _CACHE[key]


def kernel(**inputs) -> np.ndarray:
    in_maps, meta = prep_inputs(inputs, DIMS)
    nc = get_program(meta)
    res = run_bass_kernel_spmd(nc, in_maps, core_ids=list(range(N_CORES)))
    out = np.concatenate(
        [
            np.asarray(res.results[c]["out"], dtype=np.float32).reshape(GPC)
            for c in range(N_CORES)
        ]
    )
    # undo the graph bin-packing permutation (position j holds graph gperm[j])
    out_orig = np.empty_like(out)
    out_orig[np.asarray(meta["gperm"])] = out
    return out_orig[:, None]
